# revision 28
# baseline (speedup 1.0000x reference)
"""NeRF MLP forward pass on 8 Trainium2 NeuronCores (Bass/Tile), fp8 edition.

Strategy: pure data parallel over rays (512 rays x 64 samples = 32768 points
per core, sample-major). All heavy matmuls run in fp8(e4m3) with the
DoubleRow perf mode, which contracts K=256 (two 128-row k-tiles packed along
a free dim) at 0.5 PE cycles per output column. PSUM accumulates in fp32.
The bottleneck is eviction work (relu+bias+fp8-quantize of every layer
output, PSUM->SBUF): neither the Pool(GPSIMD) engine nor DMA can touch PSUM
on TRN2, so evictions are split across ACT and DVE by a weighted rotation
matched to their per-op speeds (ACT 1038ns vs DVE 1192ns per [128,1024]
eviction, ~53:47), keeping both engines ~86% busy -- the eviction-engine
pair IS the roofline for this kernel.

Harmonic embeddings: the host ships pre-folded FRACTIONAL phases
F = frac(x*f/2pi + phase) as fp16 (affine fold + range reduction of the
input; F in [-0.5, 0.5], fp16 error ~8e-4, far below the fp8 noise floor).
On-chip, the otherwise-idle Pool engine evaluates sin(2pi*F) as a degree-7
odd minimax polynomial (max err 6e-4) writing fp8 directly; a DMA shuffle
packs it into the [32,2,2048] k-tile layout DoubleRow needs. Keeping Sin
off ACT leaves Relu+Sigmoid as the only steady-state ACT functions -- both
live in one activation table, so after a startup dummy-Sigmoid forces that
table load, the eviction stream runs with ZERO table switches, and the rgb
head is a single ACT Sigmoid reading PSUM. The first super-tile's sin runs
on the startup-idle ACT engine instead (trig table, loaded before the
switch) to cut pipeline-fill latency, and dummy DoubleRow matmuls warm the
PE clock out of its cold p-state during the fill. The per-ray direction
embedding is computed once (same Pool polynomial) and reused as the
DoubleRow rhs of every sub-tile (sample-major order means every 512-point
sub-tile sees the same 512 rays in the same order).

Schedule: per 2048-point super-tile, the 8 xyz layers emit as a generator
"head" while the tail (density / feat / dir / rgb heads) is emitted
interleaved into the NEXT super-tile's head, so tail dependency stalls
don't head-block the in-order engine queues (activations double-buffered).
Queue discipline is what the layout lives or dies by: input DMAs (P/E
shuffle) ride the SP queue, weights ride ACT, and output DMAs are emitted
on SP only at the point where their data is already written -- a DMA whose
data isn't ready blocks its whole in-order queue (and the single 625ns/DMA
HWDGE descriptor generator behind it). Deep tile pools (P fp16 x16,
Sx x6, E x5) keep write-after-read recycling of the embedding pipeline off
the SP queue head. Density/rgb heads pack their 4 sub-tile outputs into
one PSUM bank at partition offsets 0/32/64/96 (plain-fp8 matmuls +
tile_position; DoubleRow forbids tile_position), evict with single
[<=99,512] ops, and only the meaningful partitions (4 density rows, 12 rgb
rows) DMA to DRAM via partition-strided access patterns.
"""

import sys

if '/opt/trn_rl_repo' not in sys.path:
    sys.path.insert(0, '/opt/trn_rl_repo')

import numpy as np
import ml_dtypes

import concourse.bacc as bacc
import concourse.mybir as mybir
import concourse.tile as tile
from concourse.bass_utils import run_bass_kernel_spmd

F32 = mybir.dt.float32
F16 = mybir.dt.float16
FP8 = mybir.dt.float8e4
NP8 = ml_dtypes.float8_e4m3
AF = mybir.ActivationFunctionType
ALU = mybir.AluOpType
DR = mybir.MatmulPerfMode.DoubleRow

N_CORES = 8
N_RAYS, S = 4096, 64
R_CORE = N_RAYS // N_CORES            # 512 rays per core
NPTS = R_CORE * S                     # 32768 points per core
# Sample-major point order: point index = s * R_CORE + r, so a 512-point
# sub-tile is one sample across all rays and the direction embedding is
# identical for every sub-tile.
F = 512                               # points per matmul (one PSUM bank)
FSUP = 2048                           # points per super-tile
NSUB = FSUP // F                      # 4
NSUP = NPTS // FSUP                   # 16
S_SUP = FSUP // R_CORE                # 4 samples per super-tile
HALF = 1024                           # embedding pipeline column block

# sin(2*pi*x) ~= x*(C1 + C2*t + C3*t^2 + C4*t^3), t = x^2, x in [-0.5, 0.5]
# (degree-7 odd minimax, max abs err 6.0e-4 -- far below fp8 e4m3 noise)
C1 = 6.28110742
C2 = -41.17292474
C3 = 78.58984384
C4 = -57.66077642
TWO_PI = float(2.0 * np.pi)

_cache = {}


def _rot_seq(n, wa, wd):
    """Weighted largest-remainder interleave of ('A','D') engines."""
    targets = {"A": float(wa), "D": float(wd)}
    tot = sum(targets.values())
    acc = {k: 0.0 for k in targets}
    seq = []
    for _ in range(n):
        for k in targets:
            acc[k] += targets[k] / tot
        pick = max(acc, key=lambda k: acc[k])
        acc[pick] -= 1.0
        seq.append(pick)
    return seq


def _build(nsup_exec=NSUP):
    key = ("nc", nsup_exec)
    if key in _cache:
        return _cache[key]

    nc = bacc.Bacc("TRN2", target_bir_lowering=False, debug=False,
                   num_devices=N_CORES)

    # Fractional harmonic phases, fp16: for super-tile st, rows 0-59 are the
    # 60 harmonic rows for points [st*2048, st*2048+1024), rows 64-123 for
    # the next 1024 points; rows 60-63 / 124-127 are zero.
    pts16 = nc.dram_tensor("pts16", [128, NSUP, HALF], F16,
                           kind="ExternalInput")
    # xyz in fp8 (+ zero pad row) for the E k-tile slots 28-31, prequantized
    # host-side so it DMAs straight into E with no cast op
    pts8 = nc.dram_tensor("pts8", [4, NPTS], FP8, kind="ExternalInput")
    # dirs: rows 0-23 fractional phases (12 sin + 12 cos), rows 24-26 xyz
    dirs27 = nc.dram_tensor("dirs27", [27, R_CORE], F16, kind="ExternalInput")
    # all fp8 weights packed into one tensor -> one DMA (HWDGE transfers
    # serialize; ~11 separate weight DMAs would block the issuing queue for
    # ~13us at startup). Columns: wmid1..7 | wfeat | wdir | wden | wrgb on
    # all 128 partitions, then w0 | w4e (32 partitions), wdire (16).
    wall = nc.dram_tensor("wall", [128, 5728], FP8, kind="ExternalInput")
    biases = nc.dram_tensor("biases", [128, 21], F32, kind="ExternalInput")
    # Compact outputs: oden row s = density of sub-tile s; orgb row 3s+c =
    # channel c of sub-tile s; columns st*F..(st+1)*F per super-tile.
    oden = nc.dram_tensor("oden", [NSUB, NSUP * F], F32,
                          kind="ExternalOutput")
    orgb = nc.dram_tensor("orgb", [3 * NSUB, NSUP * F], F32,
                          kind="ExternalOutput")

    with tile.TileContext(nc) as tc:
        with (
            tc.tile_pool(name="wpool", bufs=1) as wpool,
            tc.tile_pool(name="spool", bufs=3) as spool,
            tc.tile_pool(name="ppool", bufs=16) as ppool,
            tc.tile_pool(name="xpool", bufs=6) as xpool,
            tc.tile_pool(name="epool", bufs=5) as epool,
            tc.tile_pool(name="apool", bufs=2) as apool,
            tc.tile_pool(name="opool", bufs=2) as opool,
            tc.tile_pool(name="psumB", bufs=4, space="PSUM") as psumB,
        ):
            # ---- direction embedding per ray (once per core) ----
            pdir = wpool.tile([24, R_CORE], F16)
            pdir3 = wpool.tile([3, R_CORE], F16)
            nc.sync.dma_start(pdir[:], dirs27[0:24, :])
            nc.sync.dma_start(pdir3[:], dirs27[24:27, :])
            tdd = wpool.tile([24, R_CORE], F32)
            udd = wpool.tile([24, R_CORE], F32)
            nc.gpsimd.tensor_tensor(tdd[:], pdir[:], pdir[:], op=ALU.mult)
            nc.gpsimd.tensor_scalar(udd[:], tdd[:], C4, C3,
                                    op0=ALU.mult, op1=ALU.add)
            nc.gpsimd.tensor_tensor(udd[:], udd[:], tdd[:], op=ALU.mult)
            nc.gpsimd.tensor_scalar(udd[:], udd[:], C2, None, op0=ALU.add)
            nc.gpsimd.tensor_tensor(udd[:], udd[:], tdd[:], op=ALU.mult)
            nc.gpsimd.tensor_scalar(udd[:], udd[:], C1, None, op0=ALU.add)
            sd = wpool.tile([24, R_CORE], FP8)
            nc.gpsimd.tensor_tensor(sd[:], udd[:], pdir[:], op=ALU.mult)
            dx8 = wpool.tile([3, R_CORE], FP8)
            nc.gpsimd.tensor_scalar(dx8[:], pdir3[:], 1.0, None, op0=ALU.mult)
            # pack k-tile layout [16, 2, R]: t0 = rows 0-15, t1 = rows 16-23
            # + xyz rows 24-26 at slots 8-10, zero pad slots 11-15.
            # Engine ops need partition base % 32 == 0, so place rows by DMA.
            embd_rays = wpool.tile([16, 2, R_CORE], FP8)
            nc.gpsimd.memset(embd_rays[:], 0.0)
            nc.sync.dma_start(embd_rays[0:16, 0, :], sd[0:16, :])
            nc.sync.dma_start(embd_rays[0:8, 1, :], sd[16:24, :])
            nc.sync.dma_start(embd_rays[8:11, 1, :], dx8[:])

            # ---- super-tile embedding pipeline (generator, interleaved) ----
            # fast=True (first super-tile only) computes sin on the
            # startup-idle ACT engine instead of the Pool polynomial chain,
            # cutting pipeline-fill latency: the trig table is loaded at t=0
            # and auto-switches to the relu+sigmoid table exactly once,
            # before the first rgb Sigmoid.
            def emb_stages(st, fast=False):
                sl = slice(st * FSUP, (st + 1) * FSUP)
                P = ppool.tile([128, HALF], F16, name="P")
                nc.sync.dma_start(P[:], pts16[:, st, :])
                yield None
                if fast:
                    Sx = xpool.tile([128, HALF], FP8, name="Sx")
                    nc.scalar.activation(Sx[0:124, :], P[0:124, :], AF.Sin,
                                         bias=0.0, scale=TWO_PI)
                    yield None
                    yield None
                    yield None
                    yield None
                else:
                    T = spool.tile([128, HALF], F32, name="T")
                    nc.gpsimd.tensor_tensor(T[:], P[:], P[:], op=ALU.mult)
                    yield None
                    U = spool.tile([128, HALF], F32, name="U")
                    nc.gpsimd.tensor_scalar(U[:], T[:], C4, C3,
                                            op0=ALU.mult, op1=ALU.add)
                    yield None
                    nc.gpsimd.tensor_tensor(U[:], U[:], T[:], op=ALU.mult)
                    yield None
                    nc.gpsimd.tensor_scalar(U[:], U[:], C2, None, op0=ALU.add)
                    nc.gpsimd.tensor_tensor(U[:], U[:], T[:], op=ALU.mult)
                    yield None
                    nc.gpsimd.tensor_scalar(U[:], U[:], C1, None, op0=ALU.add)
                    Sx = xpool.tile([128, HALF], FP8, name="Sx")
                    nc.gpsimd.tensor_tensor(Sx[:], U[:], P[:], op=ALU.mult)
                    yield None
                # E k-tile layout [32, 2, FSUP]: t0 = harmonic rows 0-31,
                # t1 = rows 32-59 + xyz rows at slots 28-30 + zero pad slot 31
                E = epool.tile([32, 2, FSUP], FP8, name="E")
                nc.sync.dma_start(E[0:32, 0, 0:HALF], Sx[0:32, :])
                nc.sync.dma_start(E[0:28, 1, 0:HALF], Sx[32:60, :])
                nc.sync.dma_start(E[0:32, 0, HALF:FSUP], Sx[64:96, :])
                nc.sync.dma_start(E[0:28, 1, HALF:FSUP], Sx[96:124, :])
                # xyz rows (fp8, prequantized host-side; row 3 = zero pad)
                nc.sync.dma_start(E[28:32, 1, :], pts8[:, sl])
                yield E

            gen0 = emb_stages(0, fast=True)
            next(gen0)

            # ---- persistent weights / constants (one packed DMA) ----
            wall_t = wpool.tile([128, 5728], FP8)
            nc.scalar.dma_start(wall_t[:], wall[:])

            def wview(lo, cols, parts=128, t=2):
                v = wall_t[0:parts, lo:lo + cols]
                return v.rearrange("p (t c) -> p t c", t=t)

            wmid_t = {i: wview(512 * (i - 1), 512) for i in range(1, 8)}
            wfeat_t = wview(3584, 512)
            wdir_t = wview(4096, 256)
            wden_t = wview(4352, 64)
            wrgb_t = wall_t[0:128, 4416:4448]
            w0_t = wview(4448, 512, parts=32)
            w4e_t = wview(4960, 512, parts=32)
            wdire_t = wview(5472, 256, parts=16)
            b_t = wpool.tile([128, 21], F32)
            nc.scalar.dma_start(b_t[:], biases[:])

            # ---- PE warm-up: the tensor engine runs at 2.6x slower clock
            # until it has ~3us of continuous execution behind it. Burn the
            # dead window while the first embedding primes with small dummy
            # matmuls on weight bytes (result never read), so the first
            # super-tile's real matmuls run at full speed.
            wu_lhs = wall_t[0:128, 0:256].rearrange("p (t c) -> p t c", t=2)
            wu_rhs = wall_t[0:128, 0:512].rearrange("p (t c) -> p t c", t=2)
            wu = psumB.tile([128, 256], F32, name="warm", tag="mm")
            for _ in range(40):
                nc.tensor.matmul(wu[:, 0:256], wu_lhs, wu_rhs,
                                 start=True, stop=True, perf_mode=DR)

            # ---- eviction engine rotation (Pool has no PSUM access, and
            # DMA cannot read PSUM either, so evictions split ACT/DVE,
            # weighted by per-op speed: ACT 1038ns vs DVE 1192ns) ----
            rot = _rot_seq(311, 1172, 1038)
            ev_i = [0]

            def evict(psum_ap, out_ap, bias_ap):
                eng = rot[ev_i[0] % len(rot)]
                ev_i[0] += 1
                if eng == "A":
                    nc.scalar.activation(out_ap, psum_ap, AF.Relu,
                                         bias=bias_ap)
                else:
                    nc.vector.tensor_scalar(out_ap, psum_ap, bias_ap, 0.0,
                                            op0=ALU.add, op1=ALU.max)

            def dr_rhs(t, sub):
                """[128, 2, F] DoubleRow rhs slice of a [128, 2, FSUP] tile."""
                return t[:, :, sub * F:(sub + 1) * F]

            # ---- main loop: the per-supertile MLP is a generator whose
            # tail stages (den/feat/dir/rgb) are emitted interleaved into the
            # NEXT supertile's layer loop, so tail dependency stalls don't
            # head-block the in-order engine queues while ready layer work
            # waits behind them. Activations are double-buffered (apool).
            def mlp_tile(st, E):
                xa = apool.tile([128, 2, FSUP], FP8, name="xa")
                xb = apool.tile([128, 2, FSUP], FP8, name="xb")
                hT = apool.tile([128, FSUP], FP8, name="hT")
                osb = opool.tile([128, F], F32, name="osb")
                rgbsb = opool.tile([128, F], F32, name="rgbsb")

                cur = None
                for li in range(8):
                    nxt = xa if li % 2 == 0 else xb
                    for m in range(2):
                        for g in range(2):
                            pt = psumB.tile([128, 2 * F], F32, name="mmps",
                                            tag="mm")
                            for s in (2 * g, 2 * g + 1):
                                o = pt[:, (s - 2 * g) * F:(s - 2 * g + 1) * F]
                                if li == 0:
                                    nc.tensor.matmul(
                                        o, w0_t[:, :, m * 128:(m + 1) * 128],
                                        dr_rhs(E, s), start=True, stop=True,
                                        perf_mode=DR)
                                elif li == 4:
                                    nc.tensor.matmul(
                                        o, wmid_t[4][:, :, m * 128:(m + 1) * 128],
                                        dr_rhs(cur, s), start=True, stop=False,
                                        perf_mode=DR)
                                    nc.tensor.matmul(
                                        o, w4e_t[:, :, m * 128:(m + 1) * 128],
                                        dr_rhs(E, s), start=False, stop=True,
                                        perf_mode=DR)
                                else:
                                    nc.tensor.matmul(
                                        o, wmid_t[li][:, :, m * 128:(m + 1) * 128],
                                        dr_rhs(cur, s), start=True, stop=True,
                                        perf_mode=DR)
                            evict(pt[:], nxt[:, m, g * HALF:(g + 1) * HALF],
                                  b_t[:, 2 * li + m:2 * li + m + 1])
                    cur = nxt
                    yield None

                # ---- tail stages 1+2: density head. The plain fp8
                # k-chunk matmuls (DoubleRow + tile_position is rejected by
                # the walrus ISA check) are spread over two stages so their
                # 213ns-per-matmul PE block doesn't delay layer psum groups
                # and starve the eviction engines. M=32 replicated weight
                # columns tile all 128 psum partitions (no uninitialized
                # gaps for the eviction).
                ptd = psumB.tile([128, 2 * F], F32, name="mmps", tag="mm")
                for s in range(2):
                    for t in range(2):
                        nc.tensor.matmul(ptd[32 * s:32 * s + 32, 0:F],
                                         wden_t[:, t, :], cur[:, t,
                                         s * F:(s + 1) * F],
                                         start=(t == 0), stop=(t == 1),
                                         tile_position=(0, 32 * s))
                yield None
                for s in range(2, NSUB):
                    for t in range(2):
                        nc.tensor.matmul(ptd[32 * s:32 * s + 32, 0:F],
                                         wden_t[:, t, :], cur[:, t,
                                         s * F:(s + 1) * F],
                                         start=(t == 0), stop=(t == 1),
                                         tile_position=(0, 32 * s))
                evict(ptd[0:97, 0:F], osb[0:97, :], b_t[0:97, 19:20])
                yield None

                # ---- tail stages 3+4: feat layer (one m-chunk per stage) --
                nxt = xa if cur is xb else xb
                for m in range(2):
                    for g in range(2):
                        pt = psumB.tile([128, 2 * F], F32, name="mmps",
                                        tag="mm")
                        for s in (2 * g, 2 * g + 1):
                            o = pt[:, (s - 2 * g) * F:(s - 2 * g + 1) * F]
                            nc.tensor.matmul(
                                o, wfeat_t[:, :, m * 128:(m + 1) * 128],
                                dr_rhs(cur, s), start=True, stop=True,
                                perf_mode=DR)
                        evict(pt[:], nxt[:, m, g * HALF:(g + 1) * HALF],
                              b_t[:, 16 + m:17 + m])
                    yield None
                cur = nxt

                # ---- tail stage 5: direction layer -> h (the per-ray
                # embedding tile is the DoubleRow rhs for every sub-tile) ----
                for g in range(2):
                    pt = psumB.tile([128, 2 * F], F32, name="mmps", tag="mm")
                    for s in (2 * g, 2 * g + 1):
                        o = pt[:, (s - 2 * g) * F:(s - 2 * g + 1) * F]
                        nc.tensor.matmul(o, wdir_t, dr_rhs(cur, s),
                                         start=True, stop=False, perf_mode=DR)
                        nc.tensor.matmul(o, wdire_t, embd_rays[:],
                                         start=False, stop=True, perf_mode=DR)
                    evict(pt[:], hT[:, g * HALF:(g + 1) * HALF],
                          b_t[:, 18:19])
                yield None

                # ---- tail stage 6: rgb head: packed matmuls + one ACT
                # Sigmoid eviction (Relu and Sigmoid share one activation
                # table, so no table reloads anywhere in the kernel) ----
                ptr = psumB.tile([128, 2 * F], F32, name="mmps", tag="mm")
                for s in range(NSUB):
                    nc.tensor.matmul(ptr[32 * s:32 * s + 32, 0:F], wrgb_t,
                                     hT[:, s * F:(s + 1) * F],
                                     start=True, stop=True,
                                     tile_position=(0, 32 * s))
                nc.scalar.activation(rgbsb[0:99, :], ptr[0:99, 0:F],
                                     AF.Sigmoid, bias=b_t[0:99, 20:21])
                # DMA only meaningful partitions; one DMA per channel (the
                # DMA engine honors a partition stride on the first AP dim
                # only, so a nested [4,3] partition pattern is not usable)
                for ch in range(3):
                    nc.sync.dma_start(orgb[ch:12:3, st * F:(st + 1) * F],
                                      rgbsb[ch:97 + ch:32, :])
                # den DMA deferred here: osb was written 5 stages ago, so
                # this never blocks the SP queue's input DMAs behind it
                nc.sync.dma_start(oden[:, st * F:(st + 1) * F],
                                  osb[0:97:32, :])
                yield None

            emb_next = None
            for r in gen0:
                if r is not None:
                    emb_next = r
            # force the relu+sigmoid table load here (startup), so the only
            # mid-stream table switch happens while PE is still filling the
            # first super-tile instead of inside the steady eviction flow
            scr = wpool.tile([1, 1], F32)
            nc.scalar.activation(scr[:], b_t[0:1, 0:1], AF.Sigmoid)

            tail_prev = None
            for sti in range(nsup_exec):
                st = sti % NSUP
                E = emb_next
                emb_gen = (emb_stages((sti + 1) % NSUP)
                           if sti + 1 < nsup_exec else None)
                g = mlp_tile(st, E)
                for li in range(8):
                    next(g)
                    if tail_prev is not None and li in (0, 1, 2, 3, 4, 5):
                        next(tail_prev, None)
                    if emb_gen is not None and 1 <= li <= 7:
                        r = next(emb_gen, None)
                        if r is not None:
                            emb_next = r
                tail_prev = g
            for _ in tail_prev:
                pass

    nc.compile()
    _cache[key] = nc
    return nc


def _prep_inputs(inputs):
    """Host-side shard + transpose + fp8 weight prep + phase folding."""
    f32 = np.float32
    f16 = np.float16
    sp = np.ascontiguousarray(inputs["sample_points"], dtype=f32)
    dirs_all = np.ascontiguousarray(inputs["directions"], dtype=f32).T  # [3,N]

    def q8(w):
        return np.ascontiguousarray(np.asarray(w, dtype=f32).astype(NP8))

    def wt(w):  # [out, in] -> [in, out]
        return np.ascontiguousarray(np.asarray(w, dtype=f32).T)

    def pack_mid(w):  # [256, K256] -> [128, 2, 256] k-tile layout
        t = wt(w)                                       # [256, 256]
        return q8(t.reshape(2, 128, t.shape[1]).transpose(1, 0, 2))

    def pack_emb(wE):  # [256out, 63in] -> [32, 2, 256]: see E layout
        t = wt(wE)                                      # [63, 256]
        arr = np.zeros((32, 2, t.shape[1]), dtype=f32)
        arr[:, 0, :] = t[0:32]
        arr[0:28, 1, :] = t[32:60]
        arr[28:31, 1, :] = t[60:63]                     # xyz rows
        return q8(arr)

    shared = {}
    wallv = np.zeros((128, 5728), dtype=NP8)
    for i in range(1, 8):
        w = np.asarray(inputs[f"Wx{i}"], dtype=f32)
        if i == 4:
            wallv[:, 1536:2048] = pack_mid(w[:, :256]).reshape(128, 512)
            wallv[0:32, 4960:5472] = pack_emb(w[:, 256:]).reshape(32, 512)
        else:
            wallv[:, 512 * (i - 1):512 * i] = pack_mid(w).reshape(128, 512)
    wallv[0:32, 4448:4960] = pack_emb(inputs["Wx0"]).reshape(32, 512)
    wallv[:, 3584:4096] = pack_mid(inputs["Wfeat"]).reshape(128, 512)
    wallv[:, 4352:4416] = np.broadcast_to(
        pack_mid(inputs["Wden"]).reshape(128, 2, 1), (128, 2, 32)
    ).reshape(128, 64)
    wd0 = np.asarray(inputs["Wd0"], dtype=f32)          # [128, 283]
    wallv[:, 4096:4352] = pack_mid(wd0[:, :256]).reshape(128, 256)
    wde = wt(wd0[:, 256:283])                           # [27, 128]
    arr = np.zeros((16, 2, 128), dtype=f32)
    arr[:, 0, :] = wde[0:16]
    arr[0:8, 1, :] = wde[16:24]
    arr[8:11, 1, :] = wde[24:27]                        # xyz rows
    wallv[0:16, 5472:5728] = q8(arr).reshape(16, 256)
    wrgb3 = wt(inputs["Wrgb"])                          # [128, 3]
    wallv[:, 4416:4448] = q8(np.concatenate(
        [np.tile(wrgb3, (1, 10)), wrgb3[:, 0:2]], axis=1))  # [128, 32]
    shared["wall"] = wallv

    bias = np.zeros((128, 21), dtype=f32)
    for li in range(8):
        b = np.asarray(inputs[f"bx{li}"], dtype=f32)
        bias[:, 2 * li] = b[:128]
        bias[:, 2 * li + 1] = b[128:]
    bias[:, 16] = np.asarray(inputs["bfeat"], dtype=f32)[:128]
    bias[:, 17] = np.asarray(inputs["bfeat"], dtype=f32)[128:]
    bias[:, 18] = np.asarray(inputs["bd0"], dtype=f32)
    for s in range(4):
        bias[32 * s, 19] = float(np.asarray(inputs["bden"], dtype=f32)[0])
        bias[32 * s:32 * s + 3, 20] = np.asarray(inputs["brgb"], dtype=f32)
    shared["biases"] = bias

    def frac(x):
        return x - np.round(x)

    in_maps = []
    for c in range(N_CORES):
        m = dict(shared)
        blk = sp[c * R_CORE:(c + 1) * R_CORE]           # [R, S, 3]
        pts = blk.transpose(2, 1, 0).reshape(3, NPTS)   # sample-major
        fr = ((2.0 ** np.arange(10)) / (2.0 * np.pi)).astype(f32)
        scaled = pts[:, None, :] * fr[None, :, None]    # [3, 10, NPTS]
        p60 = np.empty((60, NPTS), dtype=f32)
        p60[0:30] = scaled.reshape(30, NPTS)            # sin rows
        p60[30:60] = p60[0:30] + f32(0.25)              # cos rows (+1/4 turn)
        f60 = frac(p60)
        p16 = np.zeros((128, NSUP, HALF), dtype=f16)
        for st in range(NSUP):
            base = st * FSUP
            p16[0:60, st, :] = f60[:, base:base + HALF]
            p16[64:124, st, :] = f60[:, base + HALF:base + FSUP]
        m["pts16"] = p16
        p8 = np.zeros((4, NPTS), dtype=f32)
        p8[0:3] = pts
        m["pts8"] = np.ascontiguousarray(p8.astype(NP8))
        d = dirs_all[:, c * R_CORE:(c + 1) * R_CORE]    # [3, R]
        fr4 = ((2.0 ** np.arange(4)) / (2.0 * np.pi)).astype(f32)
        dscaled = (d[:, None, :] * fr4[None, :, None]).reshape(12, R_CORE)
        d27 = np.empty((27, R_CORE), dtype=f16)
        d27[0:12] = frac(dscaled)
        d27[12:24] = frac(dscaled + f32(0.25))
        d27[24:27] = d
        m["dirs27"] = d27
        in_maps.append(m)
    return in_maps


def kernel(**inputs) -> np.ndarray:
    nc = _build()
    in_maps = _prep_inputs(inputs)
    res = run_bass_kernel_spmd(nc, in_maps, core_ids=list(range(N_CORES)))
    outs = []
    for c in range(N_CORES):
        od = res.results[c]["oden"]                     # [4, NSUP*F]
        og = res.results[c]["orgb"]                     # [12, NSUP*F]
        o = np.empty((4, NPTS), dtype=np.float32)       # sample-major
        for st in range(NSUP):
            for s in range(NSUB):
                lo = st * FSUP + s * F
                o[0, lo:lo + F] = od[s, st * F:(st + 1) * F]
                o[1:4, lo:lo + F] = og[3 * s:3 * s + 3, st * F:(st + 1) * F]
        outs.append(o.reshape(4, S, R_CORE).transpose(2, 1, 0))
    return np.concatenate(outs, axis=0)


# revision 30
# speedup vs baseline: 1.0130x; 1.0130x over previous
"""NeRF MLP forward pass on 8 Trainium2 NeuronCores (Bass/Tile), fp8 edition.

Strategy: pure data parallel over rays (512 rays x 64 samples = 32768 points
per core, sample-major). All heavy matmuls run in fp8(e4m3) with the
DoubleRow perf mode, which contracts K=256 (two 128-row k-tiles packed along
a free dim) at 0.5 PE cycles per output column. PSUM accumulates in fp32.
The bottleneck is eviction work (relu+bias+fp8-quantize of every layer
output, PSUM->SBUF): neither the Pool(GPSIMD) engine nor DMA can touch PSUM
on TRN2, so evictions are split across ACT and DVE by a weighted rotation
matched to their per-op speeds (ACT 1038ns vs DVE 1192ns per [128,1024]
eviction, ~53:47), keeping both engines ~86% busy -- the eviction-engine
pair IS the roofline for this kernel.

Harmonic embeddings: the host ships pre-folded FRACTIONAL phases
F = frac(x*f/2pi + phase) as fp16 (affine fold + range reduction of the
input; F in [-0.5, 0.5], fp16 error ~8e-4, far below the fp8 noise floor).
On-chip, the otherwise-idle Pool engine evaluates sin(2pi*F) as a degree-7
odd minimax polynomial (max err 6e-4) writing fp8 directly; a DMA shuffle
packs it into the [32,2,2048] k-tile layout DoubleRow needs. Keeping Sin
off ACT leaves Relu+Sigmoid as the only steady-state ACT functions -- both
live in one activation table, so after a startup dummy-Sigmoid forces that
table load, the eviction stream runs with ZERO table switches, and the rgb
head is a single ACT Sigmoid reading PSUM. The first super-tile's sin runs
on the startup-idle ACT engine instead (trig table, loaded before the
switch) to cut pipeline-fill latency, and dummy DoubleRow matmuls warm the
PE clock out of its cold p-state during the fill. The per-ray direction
embedding is computed once (same Pool polynomial) and reused as the
DoubleRow rhs of every sub-tile (sample-major order means every 512-point
sub-tile sees the same 512 rays in the same order).

Schedule: per 2048-point super-tile, the 8 xyz layers emit as a generator
"head" while the tail (density / feat / dir / rgb heads) is emitted
interleaved into the NEXT super-tile's head, so tail dependency stalls
don't head-block the in-order engine queues (activations double-buffered).
Queue discipline is what the layout lives or dies by: input DMAs (P/E
shuffle) ride the SP queue, weights ride ACT, and output DMAs are emitted
on SP only at the point where their data is already written -- a DMA whose
data isn't ready blocks its whole in-order queue (and the single 625ns/DMA
HWDGE descriptor generator behind it). Deep tile pools (P fp16 x16,
Sx x6, E x5) keep write-after-read recycling of the embedding pipeline off
the SP queue head. Density/rgb heads pack their 4 sub-tile outputs into
one PSUM bank at partition offsets 0/32/64/96 (plain-fp8 matmuls +
tile_position; DoubleRow forbids tile_position), evict with single
[<=99,512] ops, and only the meaningful partitions (4 density rows, 12 rgb
rows) DMA to DRAM via partition-strided access patterns.
"""

import sys

if '/opt/trn_rl_repo' not in sys.path:
    sys.path.insert(0, '/opt/trn_rl_repo')

import numpy as np
import ml_dtypes

import concourse.bacc as bacc
import concourse.mybir as mybir
import concourse.tile as tile
from concourse.bass_utils import run_bass_kernel_spmd

F32 = mybir.dt.float32
F16 = mybir.dt.float16
FP8 = mybir.dt.float8e4
NP8 = ml_dtypes.float8_e4m3
AF = mybir.ActivationFunctionType
ALU = mybir.AluOpType
DR = mybir.MatmulPerfMode.DoubleRow

N_CORES = 8
N_RAYS, S = 4096, 64
R_CORE = N_RAYS // N_CORES            # 512 rays per core
NPTS = R_CORE * S                     # 32768 points per core
# Sample-major point order: point index = s * R_CORE + r, so a 512-point
# sub-tile is one sample across all rays and the direction embedding is
# identical for every sub-tile.
F = 512                               # points per matmul (one PSUM bank)
FSUP = 2048                           # points per super-tile
NSUB = FSUP // F                      # 4
NSUP = NPTS // FSUP                   # 16
S_SUP = FSUP // R_CORE                # 4 samples per super-tile
HALF = 1024                           # embedding pipeline column block

# sin(2*pi*x) ~= x*(C1 + C2*t + C3*t^2 + C4*t^3), t = x^2, x in [-0.5, 0.5]
# (degree-7 odd minimax, max abs err 6.0e-4 -- far below fp8 e4m3 noise)
C1 = 6.28110742
C2 = -41.17292474
C3 = 78.58984384
C4 = -57.66077642
TWO_PI = float(2.0 * np.pi)

_cache = {}


def _rot_seq(n, wa, wd):
    """Weighted largest-remainder interleave of ('A','D') engines."""
    targets = {"A": float(wa), "D": float(wd)}
    tot = sum(targets.values())
    acc = {k: 0.0 for k in targets}
    seq = []
    for _ in range(n):
        for k in targets:
            acc[k] += targets[k] / tot
        pick = max(acc, key=lambda k: acc[k])
        acc[pick] -= 1.0
        seq.append(pick)
    return seq


def _build(nsup_exec=NSUP):
    key = ("nc", nsup_exec)
    if key in _cache:
        return _cache[key]

    nc = bacc.Bacc("TRN2", target_bir_lowering=False, debug=False,
                   num_devices=N_CORES)

    # Fractional harmonic phases, fp16: for super-tile st, rows 0-59 are the
    # 60 harmonic rows for points [st*2048, st*2048+1024), rows 64-123 for
    # the next 1024 points; rows 60-63 / 124-127 are zero.
    pts16 = nc.dram_tensor("pts16", [128, NSUP, HALF], F16,
                           kind="ExternalInput")
    # xyz in fp8 (+ zero pad row) for the E k-tile slots 28-31, prequantized
    # host-side so it DMAs straight into E with no cast op
    pts8 = nc.dram_tensor("pts8", [4, NPTS], FP8, kind="ExternalInput")
    # dirs: rows 0-23 fractional phases (12 sin + 12 cos), rows 24-26 xyz
    dirs27 = nc.dram_tensor("dirs27", [27, R_CORE], F16, kind="ExternalInput")
    # all fp8 weights packed into one tensor -> one DMA (HWDGE transfers
    # serialize; ~11 separate weight DMAs would block the issuing queue for
    # ~13us at startup). Columns: wmid1..7 | wfeat | wdir | wden | wrgb on
    # all 128 partitions, then w0 | w4e (32 partitions), wdire (16).
    wall = nc.dram_tensor("wall", [128, 5728], FP8, kind="ExternalInput")
    biases = nc.dram_tensor("biases", [128, 21], F32, kind="ExternalInput")
    # Compact outputs: oden row s = density of sub-tile s; orgb row 3s+c =
    # channel c of sub-tile s; columns st*F..(st+1)*F per super-tile.
    oden = nc.dram_tensor("oden", [NSUB, NSUP * F], F32,
                          kind="ExternalOutput")
    orgb = nc.dram_tensor("orgb", [3 * NSUB, NSUP * F], F32,
                          kind="ExternalOutput")

    with tile.TileContext(nc) as tc:
        with (
            tc.tile_pool(name="wpool", bufs=1) as wpool,
            tc.tile_pool(name="spool", bufs=3) as spool,
            tc.tile_pool(name="ppool", bufs=16) as ppool,
            tc.tile_pool(name="xpool", bufs=6) as xpool,
            tc.tile_pool(name="epool", bufs=5) as epool,
            tc.tile_pool(name="apool", bufs=2) as apool,
            tc.tile_pool(name="opool", bufs=2) as opool,
            tc.tile_pool(name="psumB", bufs=4, space="PSUM") as psumB,
        ):
            # ---- direction embedding per ray (once per core) ----
            pdir = wpool.tile([24, R_CORE], F16)
            pdir3 = wpool.tile([3, R_CORE], F16)
            nc.sync.dma_start(pdir[:], dirs27[0:24, :])
            nc.sync.dma_start(pdir3[:], dirs27[24:27, :])
            tdd = wpool.tile([24, R_CORE], F32)
            udd = wpool.tile([24, R_CORE], F32)
            nc.gpsimd.tensor_tensor(tdd[:], pdir[:], pdir[:], op=ALU.mult)
            nc.gpsimd.tensor_scalar(udd[:], tdd[:], C4, C3,
                                    op0=ALU.mult, op1=ALU.add)
            nc.gpsimd.tensor_tensor(udd[:], udd[:], tdd[:], op=ALU.mult)
            nc.gpsimd.tensor_scalar(udd[:], udd[:], C2, None, op0=ALU.add)
            nc.gpsimd.tensor_tensor(udd[:], udd[:], tdd[:], op=ALU.mult)
            nc.gpsimd.tensor_scalar(udd[:], udd[:], C1, None, op0=ALU.add)
            sd = wpool.tile([24, R_CORE], FP8)
            nc.gpsimd.tensor_tensor(sd[:], udd[:], pdir[:], op=ALU.mult)
            dx8 = wpool.tile([3, R_CORE], FP8)
            nc.gpsimd.tensor_scalar(dx8[:], pdir3[:], 1.0, None, op0=ALU.mult)
            # pack k-tile layout [16, 2, R]: t0 = rows 0-15, t1 = rows 16-23
            # + xyz rows 24-26 at slots 8-10, zero pad slots 11-15.
            # Engine ops need partition base % 32 == 0, so place rows by DMA.
            embd_rays = wpool.tile([16, 2, R_CORE], FP8)
            nc.gpsimd.memset(embd_rays[:], 0.0)
            nc.sync.dma_start(embd_rays[0:16, 0, :], sd[0:16, :])
            nc.sync.dma_start(embd_rays[0:8, 1, :], sd[16:24, :])
            nc.sync.dma_start(embd_rays[8:11, 1, :], dx8[:])

            # ---- super-tile embedding pipeline (generator, interleaved) ----
            # fast=True (first super-tile only) computes sin on the
            # startup-idle ACT engine instead of the Pool polynomial chain,
            # cutting pipeline-fill latency: the trig table is loaded at t=0
            # and auto-switches to the relu+sigmoid table exactly once,
            # before the first rgb Sigmoid.
            def emb_stages(st, fast=False):
                sl = slice(st * FSUP, (st + 1) * FSUP)
                P = ppool.tile([128, HALF], F16, name="P")
                nc.sync.dma_start(P[:], pts16[:, st, :])
                yield None
                if fast:
                    Sx = xpool.tile([128, HALF], FP8, name="Sx")
                    nc.scalar.activation(Sx[0:124, :], P[0:124, :], AF.Sin,
                                         bias=0.0, scale=TWO_PI)
                    yield None
                    yield None
                    yield None
                    yield None
                else:
                    T = spool.tile([128, HALF], F32, name="T")
                    nc.gpsimd.tensor_tensor(T[:], P[:], P[:], op=ALU.mult)
                    yield None
                    U = spool.tile([128, HALF], F32, name="U")
                    nc.gpsimd.tensor_scalar(U[:], T[:], C4, C3,
                                            op0=ALU.mult, op1=ALU.add)
                    yield None
                    nc.gpsimd.tensor_tensor(U[:], U[:], T[:], op=ALU.mult)
                    yield None
                    nc.gpsimd.tensor_scalar(U[:], U[:], C2, None, op0=ALU.add)
                    nc.gpsimd.tensor_tensor(U[:], U[:], T[:], op=ALU.mult)
                    yield None
                    nc.gpsimd.tensor_scalar(U[:], U[:], C1, None, op0=ALU.add)
                    Sx = xpool.tile([128, HALF], FP8, name="Sx")
                    nc.gpsimd.tensor_tensor(Sx[:], U[:], P[:], op=ALU.mult)
                    yield None
                # E k-tile layout [32, 2, FSUP]: t0 = harmonic rows 0-31,
                # t1 = rows 32-59 + xyz rows at slots 28-30 + zero pad slot 31
                E = epool.tile([32, 2, FSUP], FP8, name="E")
                nc.sync.dma_start(E[0:32, 0, 0:HALF], Sx[0:32, :])
                nc.sync.dma_start(E[0:28, 1, 0:HALF], Sx[32:60, :])
                nc.sync.dma_start(E[0:32, 0, HALF:FSUP], Sx[64:96, :])
                nc.sync.dma_start(E[0:28, 1, HALF:FSUP], Sx[96:124, :])
                # xyz rows (fp8, prequantized host-side; row 3 = zero pad)
                nc.sync.dma_start(E[28:32, 1, :], pts8[:, sl])
                yield E

            gen0 = emb_stages(0, fast=True)
            next(gen0)

            # ---- persistent weights / constants (one packed DMA) ----
            wall_t = wpool.tile([128, 5728], FP8)
            nc.scalar.dma_start(wall_t[:], wall[:])

            def wview(lo, cols, parts=128, t=2):
                v = wall_t[0:parts, lo:lo + cols]
                return v.rearrange("p (t c) -> p t c", t=t)

            wmid_t = {i: wview(512 * (i - 1), 512) for i in range(1, 8)}
            wfeat_t = wview(3584, 512)
            wdir_t = wview(4096, 256)
            wden_t = wview(4352, 64)
            wrgb_t = wall_t[0:128, 4416:4448]
            w0_t = wview(4448, 512, parts=32)
            w4e_t = wview(4960, 512, parts=32)
            wdire_t = wview(5472, 256, parts=16)
            b_t = wpool.tile([128, 21], F32)
            nc.scalar.dma_start(b_t[:], biases[:])

            # ---- PE warm-up: the tensor engine runs at 2.6x slower clock
            # until it has ~3us of continuous execution behind it. Burn the
            # dead window while the first embedding primes with small dummy
            # matmuls on weight bytes (result never read), so the first
            # super-tile's real matmuls run at full speed.
            wu_lhs = wall_t[0:128, 0:256].rearrange("p (t c) -> p t c", t=2)
            wu_rhs = wall_t[0:128, 0:512].rearrange("p (t c) -> p t c", t=2)
            wu = psumB.tile([128, 256], F32, name="warm", tag="mm")
            for _ in range(40):
                nc.tensor.matmul(wu[:, 0:256], wu_lhs, wu_rhs,
                                 start=True, stop=True, perf_mode=DR)

            # ---- eviction engine rotation (Pool has no PSUM access, and
            # DMA cannot read PSUM either, so evictions split ACT/DVE,
            # weighted by per-op speed: ACT 1038ns vs DVE 1192ns) ----
            rot = _rot_seq(311, 1172, 1038)
            ev_i = [0]

            def evict(psum_ap, out_ap, bias_ap):
                eng = rot[ev_i[0] % len(rot)]
                ev_i[0] += 1
                if eng == "A":
                    nc.scalar.activation(out_ap, psum_ap, AF.Relu,
                                         bias=bias_ap)
                else:
                    nc.vector.tensor_scalar(out_ap, psum_ap, bias_ap, 0.0,
                                            op0=ALU.add, op1=ALU.max)

            def dr_rhs(t, sub):
                """[128, 2, F] DoubleRow rhs slice of a [128, 2, FSUP] tile."""
                return t[:, :, sub * F:(sub + 1) * F]

            # ---- main loop: the per-supertile MLP is a generator whose
            # tail stages (den/feat/dir/rgb) are emitted interleaved into the
            # NEXT supertile's layer loop, so tail dependency stalls don't
            # head-block the in-order engine queues while ready layer work
            # waits behind them. Activations are double-buffered (apool).
            def mlp_tile(st, E):
                xa = apool.tile([128, 2, FSUP], FP8, name="xa")
                xb = apool.tile([128, 2, FSUP], FP8, name="xb")
                hT = apool.tile([128, FSUP], FP8, name="hT")
                osb = opool.tile([128, F], F32, name="osb")
                rgbsb = opool.tile([128, F], F32, name="rgbsb")

                cur = None
                for li in range(8):
                    nxt = xa if li % 2 == 0 else xb
                    for m in range(2):
                        for g in range(2):
                            pt = psumB.tile([128, 2 * F], F32, name="mmps",
                                            tag="mm")
                            for s in (2 * g, 2 * g + 1):
                                o = pt[:, (s - 2 * g) * F:(s - 2 * g + 1) * F]
                                if li == 0:
                                    nc.tensor.matmul(
                                        o, w0_t[:, :, m * 128:(m + 1) * 128],
                                        dr_rhs(E, s), start=True, stop=True,
                                        perf_mode=DR)
                                elif li == 4:
                                    # E-part first: it depends only on E and
                                    # the freed psum bank, so it runs inside
                                    # the wait for the previous layer's
                                    # evictions instead of serializing after
                                    # them -- removes the skip-layer seam
                                    # that stalled the eviction engines
                                    nc.tensor.matmul(
                                        o, w4e_t[:, :, m * 128:(m + 1) * 128],
                                        dr_rhs(E, s), start=True, stop=False,
                                        perf_mode=DR)
                                    nc.tensor.matmul(
                                        o, wmid_t[4][:, :, m * 128:(m + 1) * 128],
                                        dr_rhs(cur, s), start=False, stop=True,
                                        perf_mode=DR)
                                else:
                                    nc.tensor.matmul(
                                        o, wmid_t[li][:, :, m * 128:(m + 1) * 128],
                                        dr_rhs(cur, s), start=True, stop=True,
                                        perf_mode=DR)
                            evict(pt[:], nxt[:, m, g * HALF:(g + 1) * HALF],
                                  b_t[:, 2 * li + m:2 * li + m + 1])
                    cur = nxt
                    yield None

                # ---- tail stages 1+2: density head. The plain fp8
                # k-chunk matmuls (DoubleRow + tile_position is rejected by
                # the walrus ISA check) are spread over two stages so their
                # 213ns-per-matmul PE block doesn't delay layer psum groups
                # and starve the eviction engines. M=32 replicated weight
                # columns tile all 128 psum partitions (no uninitialized
                # gaps for the eviction).
                ptd = psumB.tile([128, 2 * F], F32, name="mmps", tag="mm")
                for s in range(2):
                    for t in range(2):
                        nc.tensor.matmul(ptd[32 * s:32 * s + 32, 0:F],
                                         wden_t[:, t, :], cur[:, t,
                                         s * F:(s + 1) * F],
                                         start=(t == 0), stop=(t == 1),
                                         tile_position=(0, 32 * s))
                yield None
                for s in range(2, NSUB):
                    for t in range(2):
                        nc.tensor.matmul(ptd[32 * s:32 * s + 32, 0:F],
                                         wden_t[:, t, :], cur[:, t,
                                         s * F:(s + 1) * F],
                                         start=(t == 0), stop=(t == 1),
                                         tile_position=(0, 32 * s))
                evict(ptd[0:97, 0:F], osb[0:97, :], b_t[0:97, 19:20])
                yield None

                # ---- tail stages 3+4: feat layer (one m-chunk per stage) --
                nxt = xa if cur is xb else xb
                for m in range(2):
                    for g in range(2):
                        pt = psumB.tile([128, 2 * F], F32, name="mmps",
                                        tag="mm")
                        for s in (2 * g, 2 * g + 1):
                            o = pt[:, (s - 2 * g) * F:(s - 2 * g + 1) * F]
                            nc.tensor.matmul(
                                o, wfeat_t[:, :, m * 128:(m + 1) * 128],
                                dr_rhs(cur, s), start=True, stop=True,
                                perf_mode=DR)
                        evict(pt[:], nxt[:, m, g * HALF:(g + 1) * HALF],
                              b_t[:, 16 + m:17 + m])
                    yield None
                cur = nxt

                # ---- tail stage 5: direction layer -> h (the per-ray
                # embedding tile is the DoubleRow rhs for every sub-tile) ----
                for g in range(2):
                    pt = psumB.tile([128, 2 * F], F32, name="mmps", tag="mm")
                    for s in (2 * g, 2 * g + 1):
                        o = pt[:, (s - 2 * g) * F:(s - 2 * g + 1) * F]
                        nc.tensor.matmul(o, wdire_t, embd_rays[:],
                                         start=True, stop=False, perf_mode=DR)
                        nc.tensor.matmul(o, wdir_t, dr_rhs(cur, s),
                                         start=False, stop=True, perf_mode=DR)
                    evict(pt[:], hT[:, g * HALF:(g + 1) * HALF],
                          b_t[:, 18:19])
                    yield None

                # ---- tail stage 6: rgb head: packed matmuls + one ACT
                # Sigmoid eviction (Relu and Sigmoid share one activation
                # table, so no table reloads anywhere in the kernel) ----
                ptr = psumB.tile([128, 2 * F], F32, name="mmps", tag="mm")
                for s in range(NSUB):
                    nc.tensor.matmul(ptr[32 * s:32 * s + 32, 0:F], wrgb_t,
                                     hT[:, s * F:(s + 1) * F],
                                     start=True, stop=True,
                                     tile_position=(0, 32 * s))
                    if s == 1:
                        yield None
                nc.scalar.activation(rgbsb[0:99, :], ptr[0:99, 0:F],
                                     AF.Sigmoid, bias=b_t[0:99, 20:21])
                # DMA only meaningful partitions; one DMA per channel (the
                # DMA engine honors a partition stride on the first AP dim
                # only, so a nested [4,3] partition pattern is not usable)
                for ch in range(3):
                    nc.sync.dma_start(orgb[ch:12:3, st * F:(st + 1) * F],
                                      rgbsb[ch:97 + ch:32, :])
                # den DMA deferred here: osb was written 5 stages ago, so
                # this never blocks the SP queue's input DMAs behind it
                nc.sync.dma_start(oden[:, st * F:(st + 1) * F],
                                  osb[0:97:32, :])
                yield None

            emb_next = None
            for r in gen0:
                if r is not None:
                    emb_next = r
            # force the relu+sigmoid table load here (startup), so the only
            # mid-stream table switch happens while PE is still filling the
            # first super-tile instead of inside the steady eviction flow
            scr = wpool.tile([1, 1], F32)
            nc.scalar.activation(scr[:], b_t[0:1, 0:1], AF.Sigmoid)

            tail_prev = None
            for sti in range(nsup_exec):
                st = sti % NSUP
                E = emb_next
                emb_gen = (emb_stages((sti + 1) % NSUP)
                           if sti + 1 < nsup_exec else None)
                g = mlp_tile(st, E)
                for li in range(8):
                    next(g)
                    if tail_prev is not None and li in (0, 1, 2, 3, 4, 5, 6, 7):
                        next(tail_prev, None)
                    if emb_gen is not None and 1 <= li <= 7:
                        r = next(emb_gen, None)
                        if r is not None:
                            emb_next = r
                tail_prev = g
            for _ in tail_prev:
                pass

    nc.compile()
    _cache[key] = nc
    return nc


def _prep_inputs(inputs):
    """Host-side shard + transpose + fp8 weight prep + phase folding."""
    f32 = np.float32
    f16 = np.float16
    sp = np.ascontiguousarray(inputs["sample_points"], dtype=f32)
    dirs_all = np.ascontiguousarray(inputs["directions"], dtype=f32).T  # [3,N]

    def q8(w):
        return np.ascontiguousarray(np.asarray(w, dtype=f32).astype(NP8))

    def wt(w):  # [out, in] -> [in, out]
        return np.ascontiguousarray(np.asarray(w, dtype=f32).T)

    def pack_mid(w):  # [256, K256] -> [128, 2, 256] k-tile layout
        t = wt(w)                                       # [256, 256]
        return q8(t.reshape(2, 128, t.shape[1]).transpose(1, 0, 2))

    def pack_emb(wE):  # [256out, 63in] -> [32, 2, 256]: see E layout
        t = wt(wE)                                      # [63, 256]
        arr = np.zeros((32, 2, t.shape[1]), dtype=f32)
        arr[:, 0, :] = t[0:32]
        arr[0:28, 1, :] = t[32:60]
        arr[28:31, 1, :] = t[60:63]                     # xyz rows
        return q8(arr)

    shared = {}
    wallv = np.zeros((128, 5728), dtype=NP8)
    for i in range(1, 8):
        w = np.asarray(inputs[f"Wx{i}"], dtype=f32)
        if i == 4:
            wallv[:, 1536:2048] = pack_mid(w[:, :256]).reshape(128, 512)
            wallv[0:32, 4960:5472] = pack_emb(w[:, 256:]).reshape(32, 512)
        else:
            wallv[:, 512 * (i - 1):512 * i] = pack_mid(w).reshape(128, 512)
    wallv[0:32, 4448:4960] = pack_emb(inputs["Wx0"]).reshape(32, 512)
    wallv[:, 3584:4096] = pack_mid(inputs["Wfeat"]).reshape(128, 512)
    wallv[:, 4352:4416] = np.broadcast_to(
        pack_mid(inputs["Wden"]).reshape(128, 2, 1), (128, 2, 32)
    ).reshape(128, 64)
    wd0 = np.asarray(inputs["Wd0"], dtype=f32)          # [128, 283]
    wallv[:, 4096:4352] = pack_mid(wd0[:, :256]).reshape(128, 256)
    wde = wt(wd0[:, 256:283])                           # [27, 128]
    arr = np.zeros((16, 2, 128), dtype=f32)
    arr[:, 0, :] = wde[0:16]
    arr[0:8, 1, :] = wde[16:24]
    arr[8:11, 1, :] = wde[24:27]                        # xyz rows
    wallv[0:16, 5472:5728] = q8(arr).reshape(16, 256)
    wrgb3 = wt(inputs["Wrgb"])                          # [128, 3]
    wallv[:, 4416:4448] = q8(np.concatenate(
        [np.tile(wrgb3, (1, 10)), wrgb3[:, 0:2]], axis=1))  # [128, 32]
    shared["wall"] = wallv

    bias = np.zeros((128, 21), dtype=f32)
    for li in range(8):
        b = np.asarray(inputs[f"bx{li}"], dtype=f32)
        bias[:, 2 * li] = b[:128]
        bias[:, 2 * li + 1] = b[128:]
    bias[:, 16] = np.asarray(inputs["bfeat"], dtype=f32)[:128]
    bias[:, 17] = np.asarray(inputs["bfeat"], dtype=f32)[128:]
    bias[:, 18] = np.asarray(inputs["bd0"], dtype=f32)
    for s in range(4):
        bias[32 * s, 19] = float(np.asarray(inputs["bden"], dtype=f32)[0])
        bias[32 * s:32 * s + 3, 20] = np.asarray(inputs["brgb"], dtype=f32)
    shared["biases"] = bias

    def frac(x):
        return x - np.round(x)

    in_maps = []
    for c in range(N_CORES):
        m = dict(shared)
        blk = sp[c * R_CORE:(c + 1) * R_CORE]           # [R, S, 3]
        pts = blk.transpose(2, 1, 0).reshape(3, NPTS)   # sample-major
        fr = ((2.0 ** np.arange(10)) / (2.0 * np.pi)).astype(f32)
        scaled = pts[:, None, :] * fr[None, :, None]    # [3, 10, NPTS]
        p60 = np.empty((60, NPTS), dtype=f32)
        p60[0:30] = scaled.reshape(30, NPTS)            # sin rows
        p60[30:60] = p60[0:30] + f32(0.25)              # cos rows (+1/4 turn)
        f60 = frac(p60)
        p16 = np.zeros((128, NSUP, HALF), dtype=f16)
        for st in range(NSUP):
            base = st * FSUP
            p16[0:60, st, :] = f60[:, base:base + HALF]
            p16[64:124, st, :] = f60[:, base + HALF:base + FSUP]
        m["pts16"] = p16
        p8 = np.zeros((4, NPTS), dtype=f32)
        p8[0:3] = pts
        m["pts8"] = np.ascontiguousarray(p8.astype(NP8))
        d = dirs_all[:, c * R_CORE:(c + 1) * R_CORE]    # [3, R]
        fr4 = ((2.0 ** np.arange(4)) / (2.0 * np.pi)).astype(f32)
        dscaled = (d[:, None, :] * fr4[None, :, None]).reshape(12, R_CORE)
        d27 = np.empty((27, R_CORE), dtype=f16)
        d27[0:12] = frac(dscaled)
        d27[12:24] = frac(dscaled + f32(0.25))
        d27[24:27] = d
        m["dirs27"] = d27
        in_maps.append(m)
    return in_maps


def kernel(**inputs) -> np.ndarray:
    nc = _build()
    in_maps = _prep_inputs(inputs)
    res = run_bass_kernel_spmd(nc, in_maps, core_ids=list(range(N_CORES)))
    outs = []
    for c in range(N_CORES):
        od = res.results[c]["oden"]                     # [4, NSUP*F]
        og = res.results[c]["orgb"]                     # [12, NSUP*F]
        o = np.empty((4, NPTS), dtype=np.float32)       # sample-major
        for st in range(NSUP):
            for s in range(NSUB):
                lo = st * FSUP + s * F
                o[0, lo:lo + F] = od[s, st * F:(st + 1) * F]
                o[1:4, lo:lo + F] = og[3 * s:3 * s + 3, st * F:(st + 1) * F]
        outs.append(o.reshape(4, S, R_CORE).transpose(2, 1, 0))
    return np.concatenate(outs, axis=0)


# revision 32
# speedup vs baseline: 1.0554x; 1.0419x over previous
"""NeRF MLP forward pass on 8 Trainium2 NeuronCores (Bass/Tile), fp8 edition.

Strategy: pure data parallel over rays (512 rays x 64 samples = 32768 points
per core, sample-major). All heavy matmuls run in fp8(e4m3) with the
DoubleRow perf mode, which contracts K=256 (two 128-row k-tiles packed along
a free dim) at 0.5 PE cycles per output column. PSUM accumulates in fp32.
The bottleneck is eviction work (relu+bias+fp8-quantize of every layer
output, PSUM->SBUF): neither the Pool(GPSIMD) engine nor DMA can touch PSUM
on TRN2, so evictions are split across ACT and DVE by a weighted rotation
matched to their per-op speeds (ACT 1038ns vs DVE 1192ns per [128,1024]
eviction, ~53:47), keeping both engines ~86% busy -- the eviction-engine
pair IS the roofline for this kernel.

Harmonic embeddings: the host ships pre-folded FRACTIONAL phases
F = frac(x*f/2pi + phase) as fp16 (affine fold + range reduction of the
input; F in [-0.5, 0.5], fp16 error ~8e-4, far below the fp8 noise floor).
On-chip, the otherwise-idle Pool engine evaluates sin(2pi*F) as a degree-7
odd minimax polynomial (max err 6e-4) writing fp8 directly; a DMA shuffle
packs it into the [32,2,2048] k-tile layout DoubleRow needs. Keeping Sin
off ACT leaves Relu+Sigmoid as the only steady-state ACT functions -- both
live in one activation table, so after a startup dummy-Sigmoid forces that
table load, the eviction stream runs with ZERO table switches, and the rgb
head is a single ACT Sigmoid reading PSUM. The first super-tile's sin runs
on the startup-idle ACT engine instead (trig table, loaded before the
switch) to cut pipeline-fill latency, and dummy DoubleRow matmuls warm the
PE clock out of its cold p-state during the fill. The per-ray direction
embedding is computed once (same Pool polynomial) and reused as the
DoubleRow rhs of every sub-tile (sample-major order means every 512-point
sub-tile sees the same 512 rays in the same order).

Schedule: per 2048-point super-tile, the 8 xyz layers emit as a generator
"head" while the tail (density / feat / dir / rgb heads) is emitted
interleaved into the NEXT super-tile's head -- spread as 8 small stages
across all 8 layer slots, so the tail's plain-fp8 matmul bursts and odd-
size evictions never bunch up in one slot and starve the eviction engines
(de-lumping dir/rgb alone was worth ~5us). Tail dependency stalls can't
head-block the in-order engine queues (activations double-buffered).
Queue discipline is what the layout lives or dies by: input DMAs (P/E
shuffle) ride the SP queue, weights ride ACT, and output DMAs are emitted
on SP only at the point where their data is already written -- a DMA whose
data isn't ready blocks its whole in-order queue (and the single 625ns/DMA
HWDGE descriptor generator behind it). Deep tile pools (P fp16 x16,
Sx x6, E x5) keep write-after-read recycling of the embedding pipeline off
the SP queue head. Density/rgb heads pack their 4 sub-tile outputs into
one PSUM bank at partition offsets 0/32/64/96 (plain-fp8 matmuls +
tile_position; DoubleRow forbids tile_position), evict with single
[<=99,512] ops, and only the meaningful partitions (4 density rows, 12 rgb
rows) DMA to DRAM via partition-strided access patterns.
"""

import sys

if '/opt/trn_rl_repo' not in sys.path:
    sys.path.insert(0, '/opt/trn_rl_repo')

import numpy as np
import ml_dtypes

import concourse.bacc as bacc
import concourse.mybir as mybir
import concourse.tile as tile
from concourse.bass_utils import run_bass_kernel_spmd

F32 = mybir.dt.float32
F16 = mybir.dt.float16
FP8 = mybir.dt.float8e4
NP8 = ml_dtypes.float8_e4m3
AF = mybir.ActivationFunctionType
ALU = mybir.AluOpType
DR = mybir.MatmulPerfMode.DoubleRow

N_CORES = 8
N_RAYS, S = 4096, 64
R_CORE = N_RAYS // N_CORES            # 512 rays per core
NPTS = R_CORE * S                     # 32768 points per core
# Sample-major point order: point index = s * R_CORE + r, so a 512-point
# sub-tile is one sample across all rays and the direction embedding is
# identical for every sub-tile.
F = 512                               # points per matmul (one PSUM bank)
FSUP = 2048                           # points per super-tile
NSUB = FSUP // F                      # 4
NSUP = NPTS // FSUP                   # 16
S_SUP = FSUP // R_CORE                # 4 samples per super-tile
HALF = 1024                           # embedding pipeline column block

# sin(2*pi*x) ~= x*(C1 + C2*t + C3*t^2 + C4*t^3), t = x^2, x in [-0.5, 0.5]
# (degree-7 odd minimax, max abs err 6.0e-4 -- far below fp8 e4m3 noise)
C1 = 6.28110742
C2 = -41.17292474
C3 = 78.58984384
C4 = -57.66077642
TWO_PI = float(2.0 * np.pi)

_cache = {}


def _rot_seq(n, wa, wd):
    """Weighted largest-remainder interleave of ('A','D') engines."""
    targets = {"A": float(wa), "D": float(wd)}
    tot = sum(targets.values())
    acc = {k: 0.0 for k in targets}
    seq = []
    for _ in range(n):
        for k in targets:
            acc[k] += targets[k] / tot
        pick = max(acc, key=lambda k: acc[k])
        acc[pick] -= 1.0
        seq.append(pick)
    return seq


def _build(nsup_exec=NSUP):
    key = ("nc", nsup_exec)
    if key in _cache:
        return _cache[key]

    nc = bacc.Bacc("TRN2", target_bir_lowering=False, debug=False,
                   num_devices=N_CORES)

    # Fractional harmonic phases, fp16: for super-tile st, rows 0-59 are the
    # 60 harmonic rows for points [st*2048, st*2048+1024), rows 64-123 for
    # the next 1024 points; rows 60-63 / 124-127 are zero.
    pts16 = nc.dram_tensor("pts16", [128, NSUP, HALF], F16,
                           kind="ExternalInput")
    # xyz in fp8 (+ zero pad row) for the E k-tile slots 28-31, prequantized
    # host-side so it DMAs straight into E with no cast op
    pts8 = nc.dram_tensor("pts8", [4, NPTS], FP8, kind="ExternalInput")
    # dirs: rows 0-23 fractional phases (12 sin + 12 cos), rows 24-26 xyz
    dirs27 = nc.dram_tensor("dirs27", [27, R_CORE], F16, kind="ExternalInput")
    # all fp8 weights packed into one tensor -> one DMA (HWDGE transfers
    # serialize; ~11 separate weight DMAs would block the issuing queue for
    # ~13us at startup). Columns: wmid1..7 | wfeat | wdir | wden | wrgb on
    # all 128 partitions, then w0 | w4e (32 partitions), wdire (16).
    wall = nc.dram_tensor("wall", [128, 5728], FP8, kind="ExternalInput")
    biases = nc.dram_tensor("biases", [128, 21], F32, kind="ExternalInput")
    # Compact outputs: oden row s = density of sub-tile s; orgb row 3s+c =
    # channel c of sub-tile s; columns st*F..(st+1)*F per super-tile.
    oden = nc.dram_tensor("oden", [NSUB, NSUP * F], F32,
                          kind="ExternalOutput")
    orgb = nc.dram_tensor("orgb", [3 * NSUB, NSUP * F], F32,
                          kind="ExternalOutput")

    with tile.TileContext(nc) as tc:
        with (
            tc.tile_pool(name="wpool", bufs=1) as wpool,
            tc.tile_pool(name="spool", bufs=3) as spool,
            tc.tile_pool(name="ppool", bufs=16) as ppool,
            tc.tile_pool(name="xpool", bufs=6) as xpool,
            tc.tile_pool(name="epool", bufs=5) as epool,
            tc.tile_pool(name="apool", bufs=2) as apool,
            tc.tile_pool(name="opool", bufs=2) as opool,
            tc.tile_pool(name="psumB", bufs=4, space="PSUM") as psumB,
        ):
            # ---- direction embedding per ray (once per core) ----
            pdir = wpool.tile([24, R_CORE], F16)
            pdir3 = wpool.tile([3, R_CORE], F16)
            nc.sync.dma_start(pdir[:], dirs27[0:24, :])
            nc.sync.dma_start(pdir3[:], dirs27[24:27, :])
            tdd = wpool.tile([24, R_CORE], F32)
            udd = wpool.tile([24, R_CORE], F32)
            nc.gpsimd.tensor_tensor(tdd[:], pdir[:], pdir[:], op=ALU.mult)
            nc.gpsimd.tensor_scalar(udd[:], tdd[:], C4, C3,
                                    op0=ALU.mult, op1=ALU.add)
            nc.gpsimd.tensor_tensor(udd[:], udd[:], tdd[:], op=ALU.mult)
            nc.gpsimd.tensor_scalar(udd[:], udd[:], C2, None, op0=ALU.add)
            nc.gpsimd.tensor_tensor(udd[:], udd[:], tdd[:], op=ALU.mult)
            nc.gpsimd.tensor_scalar(udd[:], udd[:], C1, None, op0=ALU.add)
            sd = wpool.tile([24, R_CORE], FP8)
            nc.gpsimd.tensor_tensor(sd[:], udd[:], pdir[:], op=ALU.mult)
            dx8 = wpool.tile([3, R_CORE], FP8)
            nc.gpsimd.tensor_scalar(dx8[:], pdir3[:], 1.0, None, op0=ALU.mult)
            # pack k-tile layout [16, 2, R]: t0 = rows 0-15, t1 = rows 16-23
            # + xyz rows 24-26 at slots 8-10, zero pad slots 11-15.
            # Engine ops need partition base % 32 == 0, so place rows by DMA.
            embd_rays = wpool.tile([16, 2, R_CORE], FP8)
            nc.gpsimd.memset(embd_rays[:], 0.0)
            nc.sync.dma_start(embd_rays[0:16, 0, :], sd[0:16, :])
            nc.sync.dma_start(embd_rays[0:8, 1, :], sd[16:24, :])
            nc.sync.dma_start(embd_rays[8:11, 1, :], dx8[:])

            # ---- super-tile embedding pipeline (generator, interleaved) ----
            # fast=True (first super-tile only) computes sin on the
            # startup-idle ACT engine instead of the Pool polynomial chain,
            # cutting pipeline-fill latency: the trig table is loaded at t=0
            # and auto-switches to the relu+sigmoid table exactly once,
            # before the first rgb Sigmoid.
            def emb_stages(st, fast=False):
                sl = slice(st * FSUP, (st + 1) * FSUP)
                P = ppool.tile([128, HALF], F16, name="P")
                nc.sync.dma_start(P[:], pts16[:, st, :])
                yield None
                if fast:
                    Sx = xpool.tile([128, HALF], FP8, name="Sx")
                    nc.scalar.activation(Sx[0:124, :], P[0:124, :], AF.Sin,
                                         bias=0.0, scale=TWO_PI)
                    yield None
                    yield None
                    yield None
                    yield None
                else:
                    T = spool.tile([128, HALF], F32, name="T")
                    nc.gpsimd.tensor_tensor(T[:], P[:], P[:], op=ALU.mult)
                    yield None
                    U = spool.tile([128, HALF], F32, name="U")
                    nc.gpsimd.tensor_scalar(U[:], T[:], C4, C3,
                                            op0=ALU.mult, op1=ALU.add)
                    yield None
                    nc.gpsimd.tensor_tensor(U[:], U[:], T[:], op=ALU.mult)
                    yield None
                    nc.gpsimd.tensor_scalar(U[:], U[:], C2, None, op0=ALU.add)
                    nc.gpsimd.tensor_tensor(U[:], U[:], T[:], op=ALU.mult)
                    yield None
                    nc.gpsimd.tensor_scalar(U[:], U[:], C1, None, op0=ALU.add)
                    Sx = xpool.tile([128, HALF], FP8, name="Sx")
                    nc.gpsimd.tensor_tensor(Sx[:], U[:], P[:], op=ALU.mult)
                    yield None
                # E k-tile layout [32, 2, FSUP]: t0 = harmonic rows 0-31,
                # t1 = rows 32-59 + xyz rows at slots 28-30 + zero pad slot 31
                E = epool.tile([32, 2, FSUP], FP8, name="E")
                nc.sync.dma_start(E[0:32, 0, 0:HALF], Sx[0:32, :])
                nc.sync.dma_start(E[0:28, 1, 0:HALF], Sx[32:60, :])
                nc.sync.dma_start(E[0:32, 0, HALF:FSUP], Sx[64:96, :])
                nc.sync.dma_start(E[0:28, 1, HALF:FSUP], Sx[96:124, :])
                # xyz rows (fp8, prequantized host-side; row 3 = zero pad)
                nc.sync.dma_start(E[28:32, 1, :], pts8[:, sl])
                yield E

            gen0 = emb_stages(0, fast=True)
            next(gen0)

            # ---- persistent weights / constants (one packed DMA) ----
            wall_t = wpool.tile([128, 5728], FP8)
            nc.scalar.dma_start(wall_t[:], wall[:])

            def wview(lo, cols, parts=128, t=2):
                v = wall_t[0:parts, lo:lo + cols]
                return v.rearrange("p (t c) -> p t c", t=t)

            wmid_t = {i: wview(512 * (i - 1), 512) for i in range(1, 8)}
            wfeat_t = wview(3584, 512)
            wdir_t = wview(4096, 256)
            wden_t = wview(4352, 64)
            wrgb_t = wall_t[0:128, 4416:4448]
            w0_t = wview(4448, 512, parts=32)
            w4e_t = wview(4960, 512, parts=32)
            wdire_t = wview(5472, 256, parts=16)
            b_t = wpool.tile([128, 21], F32)
            nc.scalar.dma_start(b_t[:], biases[:])

            # ---- PE warm-up: the tensor engine runs at 2.6x slower clock
            # until it has ~3us of continuous execution behind it. Burn the
            # dead window while the first embedding primes with small dummy
            # matmuls on weight bytes (result never read), so the first
            # super-tile's real matmuls run at full speed.
            wu_lhs = wall_t[0:128, 0:256].rearrange("p (t c) -> p t c", t=2)
            wu_rhs = wall_t[0:128, 0:512].rearrange("p (t c) -> p t c", t=2)
            wu = psumB.tile([128, 256], F32, name="warm", tag="mm")
            for _ in range(40):
                nc.tensor.matmul(wu[:, 0:256], wu_lhs, wu_rhs,
                                 start=True, stop=True, perf_mode=DR)

            # ---- eviction engine rotation (Pool has no PSUM access, and
            # DMA cannot read PSUM either, so evictions split ACT/DVE,
            # weighted by per-op speed: ACT 1038ns vs DVE 1192ns) ----
            rot = _rot_seq(311, 1172, 1038)
            ev_i = [0]

            def evict(psum_ap, out_ap, bias_ap):
                eng = rot[ev_i[0] % len(rot)]
                ev_i[0] += 1
                if eng == "A":
                    nc.scalar.activation(out_ap, psum_ap, AF.Relu,
                                         bias=bias_ap)
                else:
                    nc.vector.tensor_scalar(out_ap, psum_ap, bias_ap, 0.0,
                                            op0=ALU.add, op1=ALU.max)

            def dr_rhs(t, sub):
                """[128, 2, F] DoubleRow rhs slice of a [128, 2, FSUP] tile."""
                return t[:, :, sub * F:(sub + 1) * F]

            # ---- main loop: the per-supertile MLP is a generator whose
            # tail stages (den/feat/dir/rgb) are emitted interleaved into the
            # NEXT supertile's layer loop, so tail dependency stalls don't
            # head-block the in-order engine queues while ready layer work
            # waits behind them. Activations are double-buffered (apool).
            def mlp_tile(st, E):
                xa = apool.tile([128, 2, FSUP], FP8, name="xa")
                xb = apool.tile([128, 2, FSUP], FP8, name="xb")
                hT = apool.tile([128, FSUP], FP8, name="hT")
                osb = opool.tile([128, F], F32, name="osb")
                rgbsb = opool.tile([128, F], F32, name="rgbsb")

                cur = None
                for li in range(8):
                    nxt = xa if li % 2 == 0 else xb
                    for g in range(2):
                        for m in range(2):
                            pt = psumB.tile([128, 2 * F], F32, name="mmps",
                                            tag="mm")
                            for s in (2 * g, 2 * g + 1):
                                o = pt[:, (s - 2 * g) * F:(s - 2 * g + 1) * F]
                                if li == 0:
                                    nc.tensor.matmul(
                                        o, w0_t[:, :, m * 128:(m + 1) * 128],
                                        dr_rhs(E, s), start=True, stop=True,
                                        perf_mode=DR)
                                elif li == 4:
                                    # E-part first: it depends only on E and
                                    # the freed psum bank, so it runs inside
                                    # the wait for the previous layer's
                                    # evictions instead of serializing after
                                    # them -- removes the skip-layer seam
                                    # that stalled the eviction engines
                                    nc.tensor.matmul(
                                        o, w4e_t[:, :, m * 128:(m + 1) * 128],
                                        dr_rhs(E, s), start=True, stop=False,
                                        perf_mode=DR)
                                    nc.tensor.matmul(
                                        o, wmid_t[4][:, :, m * 128:(m + 1) * 128],
                                        dr_rhs(cur, s), start=False, stop=True,
                                        perf_mode=DR)
                                else:
                                    nc.tensor.matmul(
                                        o, wmid_t[li][:, :, m * 128:(m + 1) * 128],
                                        dr_rhs(cur, s), start=True, stop=True,
                                        perf_mode=DR)
                            evict(pt[:], nxt[:, m, g * HALF:(g + 1) * HALF],
                                  b_t[:, 2 * li + m:2 * li + m + 1])
                    cur = nxt
                    yield None

                # ---- tail stages 1+2: density head. The plain fp8
                # k-chunk matmuls (DoubleRow + tile_position is rejected by
                # the walrus ISA check) are spread over two stages so their
                # 213ns-per-matmul PE block doesn't delay layer psum groups
                # and starve the eviction engines. M=32 replicated weight
                # columns tile all 128 psum partitions (no uninitialized
                # gaps for the eviction).
                ptd = psumB.tile([128, 2 * F], F32, name="mmps", tag="mm")
                for s in range(2):
                    for t in range(2):
                        nc.tensor.matmul(ptd[32 * s:32 * s + 32, 0:F],
                                         wden_t[:, t, :], cur[:, t,
                                         s * F:(s + 1) * F],
                                         start=(t == 0), stop=(t == 1),
                                         tile_position=(0, 32 * s))
                yield None
                for s in range(2, NSUB):
                    for t in range(2):
                        nc.tensor.matmul(ptd[32 * s:32 * s + 32, 0:F],
                                         wden_t[:, t, :], cur[:, t,
                                         s * F:(s + 1) * F],
                                         start=(t == 0), stop=(t == 1),
                                         tile_position=(0, 32 * s))
                evict(ptd[0:97, 0:F], osb[0:97, :], b_t[0:97, 19:20])
                yield None

                # ---- tail stages 3+4: feat layer (one m-chunk per stage) --
                nxt = xa if cur is xb else xb
                for g in range(2):
                    for m in range(2):
                        pt = psumB.tile([128, 2 * F], F32, name="mmps",
                                        tag="mm")
                        for s in (2 * g, 2 * g + 1):
                            o = pt[:, (s - 2 * g) * F:(s - 2 * g + 1) * F]
                            nc.tensor.matmul(
                                o, wfeat_t[:, :, m * 128:(m + 1) * 128],
                                dr_rhs(cur, s), start=True, stop=True,
                                perf_mode=DR)
                        evict(pt[:], nxt[:, m, g * HALF:(g + 1) * HALF],
                              b_t[:, 16 + m:17 + m])
                    yield None
                cur = nxt

                # ---- tail stage 5: direction layer -> h (the per-ray
                # embedding tile is the DoubleRow rhs for every sub-tile) ----
                for g in range(2):
                    pt = psumB.tile([128, 2 * F], F32, name="mmps", tag="mm")
                    for s in (2 * g, 2 * g + 1):
                        o = pt[:, (s - 2 * g) * F:(s - 2 * g + 1) * F]
                        nc.tensor.matmul(o, wdire_t, embd_rays[:],
                                         start=True, stop=False, perf_mode=DR)
                        nc.tensor.matmul(o, wdir_t, dr_rhs(cur, s),
                                         start=False, stop=True, perf_mode=DR)
                    evict(pt[:], hT[:, g * HALF:(g + 1) * HALF],
                          b_t[:, 18:19])
                    yield None

                # ---- tail stage 6: rgb head: packed matmuls + one ACT
                # Sigmoid eviction (Relu and Sigmoid share one activation
                # table, so no table reloads anywhere in the kernel) ----
                ptr = psumB.tile([128, 2 * F], F32, name="mmps", tag="mm")
                for s in range(NSUB):
                    nc.tensor.matmul(ptr[32 * s:32 * s + 32, 0:F], wrgb_t,
                                     hT[:, s * F:(s + 1) * F],
                                     start=True, stop=True,
                                     tile_position=(0, 32 * s))
                    if s == 1:
                        yield None
                nc.scalar.activation(rgbsb[0:99, :], ptr[0:99, 0:F],
                                     AF.Sigmoid, bias=b_t[0:99, 20:21])
                # DMA only meaningful partitions; one DMA per channel (the
                # DMA engine honors a partition stride on the first AP dim
                # only, so a nested [4,3] partition pattern is not usable)
                for ch in range(3):
                    nc.sync.dma_start(orgb[ch:12:3, st * F:(st + 1) * F],
                                      rgbsb[ch:97 + ch:32, :])
                # den DMA deferred here: osb was written 5 stages ago, so
                # this never blocks the SP queue's input DMAs behind it
                nc.sync.dma_start(oden[:, st * F:(st + 1) * F],
                                  osb[0:97:32, :])
                yield None

            emb_next = None
            for r in gen0:
                if r is not None:
                    emb_next = r
            # force the relu+sigmoid table load here (startup), so the only
            # mid-stream table switch happens while PE is still filling the
            # first super-tile instead of inside the steady eviction flow
            scr = wpool.tile([1, 1], F32)
            nc.scalar.activation(scr[:], b_t[0:1, 0:1], AF.Sigmoid)

            tail_prev = None
            for sti in range(nsup_exec):
                st = sti % NSUP
                E = emb_next
                emb_gen = (emb_stages((sti + 1) % NSUP)
                           if sti + 1 < nsup_exec else None)
                g = mlp_tile(st, E)
                for li in range(8):
                    next(g)
                    if tail_prev is not None and li in (0, 1, 2, 3, 4, 5, 6, 7):
                        next(tail_prev, None)
                    if emb_gen is not None and 1 <= li <= 7:
                        r = next(emb_gen, None)
                        if r is not None:
                            emb_next = r
                tail_prev = g
            for _ in tail_prev:
                pass

    nc.compile()
    _cache[key] = nc
    return nc


def _prep_inputs(inputs):
    """Host-side shard + transpose + fp8 weight prep + phase folding."""
    f32 = np.float32
    f16 = np.float16
    sp = np.ascontiguousarray(inputs["sample_points"], dtype=f32)
    dirs_all = np.ascontiguousarray(inputs["directions"], dtype=f32).T  # [3,N]

    def q8(w):
        return np.ascontiguousarray(np.asarray(w, dtype=f32).astype(NP8))

    def wt(w):  # [out, in] -> [in, out]
        return np.ascontiguousarray(np.asarray(w, dtype=f32).T)

    def pack_mid(w):  # [256, K256] -> [128, 2, 256] k-tile layout
        t = wt(w)                                       # [256, 256]
        return q8(t.reshape(2, 128, t.shape[1]).transpose(1, 0, 2))

    def pack_emb(wE):  # [256out, 63in] -> [32, 2, 256]: see E layout
        t = wt(wE)                                      # [63, 256]
        arr = np.zeros((32, 2, t.shape[1]), dtype=f32)
        arr[:, 0, :] = t[0:32]
        arr[0:28, 1, :] = t[32:60]
        arr[28:31, 1, :] = t[60:63]                     # xyz rows
        return q8(arr)

    shared = {}
    wallv = np.zeros((128, 5728), dtype=NP8)
    for i in range(1, 8):
        w = np.asarray(inputs[f"Wx{i}"], dtype=f32)
        if i == 4:
            wallv[:, 1536:2048] = pack_mid(w[:, :256]).reshape(128, 512)
            wallv[0:32, 4960:5472] = pack_emb(w[:, 256:]).reshape(32, 512)
        else:
            wallv[:, 512 * (i - 1):512 * i] = pack_mid(w).reshape(128, 512)
    wallv[0:32, 4448:4960] = pack_emb(inputs["Wx0"]).reshape(32, 512)
    wallv[:, 3584:4096] = pack_mid(inputs["Wfeat"]).reshape(128, 512)
    wallv[:, 4352:4416] = np.broadcast_to(
        pack_mid(inputs["Wden"]).reshape(128, 2, 1), (128, 2, 32)
    ).reshape(128, 64)
    wd0 = np.asarray(inputs["Wd0"], dtype=f32)          # [128, 283]
    wallv[:, 4096:4352] = pack_mid(wd0[:, :256]).reshape(128, 256)
    wde = wt(wd0[:, 256:283])                           # [27, 128]
    arr = np.zeros((16, 2, 128), dtype=f32)
    arr[:, 0, :] = wde[0:16]
    arr[0:8, 1, :] = wde[16:24]
    arr[8:11, 1, :] = wde[24:27]                        # xyz rows
    wallv[0:16, 5472:5728] = q8(arr).reshape(16, 256)
    wrgb3 = wt(inputs["Wrgb"])                          # [128, 3]
    wallv[:, 4416:4448] = q8(np.concatenate(
        [np.tile(wrgb3, (1, 10)), wrgb3[:, 0:2]], axis=1))  # [128, 32]
    shared["wall"] = wallv

    bias = np.zeros((128, 21), dtype=f32)
    for li in range(8):
        b = np.asarray(inputs[f"bx{li}"], dtype=f32)
        bias[:, 2 * li] = b[:128]
        bias[:, 2 * li + 1] = b[128:]
    bias[:, 16] = np.asarray(inputs["bfeat"], dtype=f32)[:128]
    bias[:, 17] = np.asarray(inputs["bfeat"], dtype=f32)[128:]
    bias[:, 18] = np.asarray(inputs["bd0"], dtype=f32)
    for s in range(4):
        bias[32 * s, 19] = float(np.asarray(inputs["bden"], dtype=f32)[0])
        bias[32 * s:32 * s + 3, 20] = np.asarray(inputs["brgb"], dtype=f32)
    shared["biases"] = bias

    def frac(x):
        return x - np.round(x)

    in_maps = []
    for c in range(N_CORES):
        m = dict(shared)
        blk = sp[c * R_CORE:(c + 1) * R_CORE]           # [R, S, 3]
        pts = blk.transpose(2, 1, 0).reshape(3, NPTS)   # sample-major
        fr = ((2.0 ** np.arange(10)) / (2.0 * np.pi)).astype(f32)
        scaled = pts[:, None, :] * fr[None, :, None]    # [3, 10, NPTS]
        p60 = np.empty((60, NPTS), dtype=f32)
        p60[0:30] = scaled.reshape(30, NPTS)            # sin rows
        p60[30:60] = p60[0:30] + f32(0.25)              # cos rows (+1/4 turn)
        f60 = frac(p60)
        p16 = np.zeros((128, NSUP, HALF), dtype=f16)
        for st in range(NSUP):
            base = st * FSUP
            p16[0:60, st, :] = f60[:, base:base + HALF]
            p16[64:124, st, :] = f60[:, base + HALF:base + FSUP]
        m["pts16"] = p16
        p8 = np.zeros((4, NPTS), dtype=f32)
        p8[0:3] = pts
        m["pts8"] = np.ascontiguousarray(p8.astype(NP8))
        d = dirs_all[:, c * R_CORE:(c + 1) * R_CORE]    # [3, R]
        fr4 = ((2.0 ** np.arange(4)) / (2.0 * np.pi)).astype(f32)
        dscaled = (d[:, None, :] * fr4[None, :, None]).reshape(12, R_CORE)
        d27 = np.empty((27, R_CORE), dtype=f16)
        d27[0:12] = frac(dscaled)
        d27[12:24] = frac(dscaled + f32(0.25))
        d27[24:27] = d
        m["dirs27"] = d27
        in_maps.append(m)
    return in_maps


def kernel(**inputs) -> np.ndarray:
    nc = _build()
    in_maps = _prep_inputs(inputs)
    res = run_bass_kernel_spmd(nc, in_maps, core_ids=list(range(N_CORES)))
    outs = []
    for c in range(N_CORES):
        od = res.results[c]["oden"]                     # [4, NSUP*F]
        og = res.results[c]["orgb"]                     # [12, NSUP*F]
        o = np.empty((4, NPTS), dtype=np.float32)       # sample-major
        for st in range(NSUP):
            for s in range(NSUB):
                lo = st * FSUP + s * F
                o[0, lo:lo + F] = od[s, st * F:(st + 1) * F]
                o[1:4, lo:lo + F] = og[3 * s:3 * s + 3, st * F:(st + 1) * F]
        outs.append(o.reshape(4, S, R_CORE).transpose(2, 1, 0))
    return np.concatenate(outs, axis=0)
